# revision 22
# baseline (speedup 1.0000x reference)
"""Multi-head attention (B=2, S=2048, E=1024, H=16) on 8 Trainium2 NeuronCores.

Sharding: data-parallel over the 2 batches x tensor-parallel over 4 head-groups
(4 heads each).  Core c handles batch c//4, heads [4*(c%4), 4*(c%4)+4).
Each core computes its heads' Q/K/V projections, softmax(QK^T/8)V, and the
partial output projection against the matching Wo column slice; the host sums
the 4 partials per batch (the head-parallel all-reduce) and stacks batches.

Device-side layout notes:
 - Host pre-transposes x and the weight slices so every matmul operand already
   has its contraction dim on SBUF partitions (no on-device transposes).
 - Scores are produced transposed, sT[j, i] = k_j . q_i, so softmax(j) is a
   partition-dim reduction folded into the P@V matmul via a ones column on V
   (out row 64 = sum_j exp(sT[j, i])), and the attention output lands directly
   in the [head_dim, seq] layout the output projection needs as lhsT.
 - exp runs on the scalar engine straight out of PSUM with the 1/sqrt(dk)
   scale and a constant -4 bias folded in (softmax is shift-invariant).
 - Matmuls use float32r (full PE rate at free-dim >= 256, ~19-bit mantissa).
"""

import numpy as np
import ml_dtypes

import concourse.bass as bass
from concourse import bacc
import concourse.mybir as mybir
import concourse.tile as tile
from concourse.bass_utils import run_bass_kernel_spmd

B, S, E, H = 2, 2048, 1024, 16
DK = 64
NCORES = 8
HGROUPS = 4            # head-parallel groups per batch
HLOC = H // HGROUPS    # heads per core = 4
FH = HLOC * DK         # local feature cols = 256

F32 = mybir.dt.float32
F32R = mybir.dt.float32r
EXP_BIAS = -4.0        # constant shift inside exp; cancels in softmax


def _round_f32r(a: np.ndarray) -> np.ndarray:
    """Round fp32 to the bf16-pair (hi+lo) values the PE's fp32r mode uses."""
    hi = a.astype(ml_dtypes.bfloat16).astype(np.float32)
    lo = (a - hi).astype(ml_dtypes.bfloat16).astype(np.float32)
    return hi + lo


def _build_program() -> bass.Bass:
    nc = bacc.Bacc("TRN2", target_bir_lowering=False, debug=False,
                   enable_asserts=False)

    xt_d = nc.dram_tensor("xt", [E, S], F32R, kind="ExternalInput").ap()
    wqt_d = nc.dram_tensor("wqt", [E, FH], F32R, kind="ExternalInput").ap()
    wkt_d = nc.dram_tensor("wkt", [E, FH], F32R, kind="ExternalInput").ap()
    wvt_d = nc.dram_tensor("wvt", [E, FH], F32R, kind="ExternalInput").ap()
    wot_d = nc.dram_tensor("wot", [FH, E], F32R, kind="ExternalInput").ap()
    ones_d = nc.dram_tensor("ones", [128, DK], F32R, kind="ExternalInput").ap()
    y_d = nc.dram_tensor("y", [S, E], F32, kind="ExternalOutput").ap()

    EC = E // 128        # 8 contraction chunks for the projections
    ST = S // 128        # 16 seq tiles of 128 (the j tiles)
    SC = S // 512        # 4 seq chunks of 512 (the i chunks)
    FT = FH // 128       # 2 feature tiles (head pairs)

    with tile.TileContext(nc) as tc:
        with (
            tc.tile_pool(name="constp", bufs=1) as constp,
            tc.tile_pool(name="xtp", bufs=1) as xtp,
            tc.tile_pool(name="wp", bufs=1) as wp,
            tc.tile_pool(name="qkp", bufs=1) as qkp,
            tc.tile_pool(name="vp", bufs=ST) as vp,
            tc.tile_pool(name="cp", bufs=3) as cp,
            tc.tile_pool(name="ep", bufs=5) as ep,
            tc.tile_pool(name="aup", bufs=6) as aup,
            tc.tile_pool(name="smp", bufs=2) as smp,
            tc.tile_pool(name="op", bufs=2) as op,
            tc.tile_pool(name="mmp", bufs=2, space="PSUM") as mmp,
            tc.tile_pool(name="scp", bufs=2, space="PSUM") as scp,
            tc.tile_pool(name="atp", bufs=1, space="PSUM") as atp,
        ):
            ones = constp.tile([128, DK], F32R, tag="ones")
            nc.sync.dma_start(ones[:], ones_d)
            bias_t = constp.tile([128, 1], F32, tag="bias")
            nc.vector.memset(bias_t[:], EXP_BIAS)
            onescol = constp.tile([128, HLOC], F32, tag="onescol")
            nc.vector.memset(onescol[:], 1.0)

            # ---- input DMAs ----
            XT = xtp.tile([128, EC, S], F32R, tag="xt")
            xt_r = xt_d.rearrange("(c p) s -> p c s", p=128)
            for ec in range(EC):
                nc.sync.dma_start(XT[:, ec, :], xt_r[:, ec, :])

            WQ = wp.tile([128, EC, FH], F32R, tag="wq")
            WK = wp.tile([128, EC, FH], F32R, tag="wk")
            WV = wp.tile([128, EC, FH], F32R, tag="wv")
            WO = wp.tile([128, FT, E], F32R, tag="wo")
            nc.sync.dma_start(WK[:], wkt_d.rearrange("(c p) f -> p c f", p=128))
            nc.sync.dma_start(WQ[:], wqt_d.rearrange("(c p) f -> p c f", p=128))
            nc.sync.dma_start(WV[:], wvt_d.rearrange("(c p) f -> p c f", p=128))
            nc.sync.dma_start(WO[:], wot_d.rearrange("(c p) e -> p c e", p=128))

            # ---- PE warmup during the input-DMA window ----
            # The HAM clock gate starts at half clock and needs ~3.4us of
            # sustained PE activity; burn cheap bf16 matmuls on junk data
            # while the 13MB of inputs stream in so the projections run warm.
            warm = constp.tile([128, 512], mybir.dt.bfloat16, tag="warm")
            nc.vector.memset(warm[:], 1.0)
            ps_w = mmp.tile([128, 512], F32, tag="mm", name="warmps")
            for _ in range(130):
                nc.tensor.matmul(ps_w[:, :], warm[:, 0:128], warm[:, :],
                                 start=True, stop=True)

            # ---- projections ----
            # qT/kT: [f, s] layout.  out[f_tile, s_chunk] = sum_ec WqT^T @ xT
            # Order: k/q for head-pair 0, then v, then k/q for pair 1, so the
            # attention loop can start as soon as pair 0 and v are ready.
            QT = qkp.tile([128, FT, S], F32R, tag="qt")
            KT = qkp.tile([128, FT, S], F32R, tag="kt")

            def qk_proj(dst, w, ft):
                for sc in range(SC):
                    ps = mmp.tile([128, 512], F32, tag="mm", name="qkps")
                    for ec in range(EC):
                        nc.tensor.matmul(
                            ps[:, :],
                            w[:, ec, ft * 128:(ft + 1) * 128],
                            XT[:, ec, sc * 512:(sc + 1) * 512],
                            start=(ec == 0), stop=(ec == EC - 1),
                        )
                    nc.vector.tensor_copy(
                        dst[:, ft, sc * 512:(sc + 1) * 512], ps[:, :])

            # v: natural [s, f] layout, plus a fused ones column per head:
            # VAUG[jt] is [128, HLOC, DK+1] with [:, h, DK] == 1.
            VAUG = []

            def v_proj():
                for jt in range(ST):
                    va = vp.tile([128, HLOC, DK + 1], F32R, tag="vaug")
                    nc.vector.tensor_copy(va[:, :, DK:DK + 1],
                                          onescol[:, :, None])
                    ps = mmp.tile([128, 512], F32, tag="mm", name="vps")
                    for ec in range(EC):
                        nc.tensor.matmul(
                            ps[:, 0:FH],
                            XT[:, ec, jt * 128:(jt + 1) * 128],
                            WV[:, ec, :],
                            start=(ec == 0), stop=(ec == EC - 1),
                        )
                    nc.vector.tensor_copy(
                        va[:, :, 0:DK],
                        ps[:, 0:FH].rearrange("p (h d) -> p h d", d=DK))
                    VAUG.append(va)

            qk_proj(KT, WK, 0)
            qk_proj(QT, WQ, 0)
            v_proj()
            qk_proj(KT, WK, 1)
            qk_proj(QT, WQ, 1)

            # ---- attention + output projection, per 512-wide i chunk ----
            # Heads are processed in pairs (partition bases 0/64) so the K=64
            # score matmuls run concurrently in distinct PE row groups.  The
            # attention PSUM is copied to SBUF right after the PV chain so the
            # slow single-partition reciprocal never holds a PSUM bank (PE
            # gaps at head boundaries re-throttle the HAM clock gate).
            # Output projection for chunk ic is emitted interleaved into the
            # NEXT chunk's jt loops so the PE never bursts at boundaries and
            # the scalar engine keeps receiving fresh score tiles.
            pending = []

            def phase_c(ic, concat):
                for stl in range(4):
                    st = ic * 4 + stl
                    for oc in range(2):
                        def emit(st=st, oc=oc, stl=stl, concat=concat):
                            ps_o = mmp.tile([128, 512], F32, tag="mm",
                                            name="ops")
                            for fc in range(FT):
                                nc.tensor.matmul(
                                    ps_o[:, :],
                                    concat[:, fc, stl * 128:(stl + 1) * 128],
                                    WO[:, fc, oc * 512:(oc + 1) * 512],
                                    start=(fc == 0), stop=(fc == FT - 1),
                                )
                            ot = op.tile([128, 512], F32, tag="out")
                            nc.vector.tensor_copy(ot[:], ps_o[:, :])
                            nc.sync.dma_start(
                                y_d[st * 128:(st + 1) * 128,
                                    oc * 512:(oc + 1) * 512],
                                ot[:])
                        pending.append(emit)

            def normalize(ic, concat, aus, dn):
                # batched reciprocal for all 4 heads of chunk ic, then four
                # broadcast-matmul + multiply pairs writing concat.
                rd = smp.tile([128, 512], F32R, tag="rd")

                def recip(rd=rd, dn=dn):
                    with nc.allow_low_precision(
                            reason="f32r softmax denominators"):
                        nc.vector.reciprocal(rd[:], dn[:])
                pending.append(recip)

                for h in range(HLOC):
                    def norm_h(h=h, rd=rd, concat=concat, au=aus[h]):
                        ft, hs = h // 2, h % 2
                        pb = hs * DK
                        ps_b = mmp.tile([DK, 512], F32, tag="mm", name="bc")
                        nc.tensor.matmul(ps_b[:, :],
                                         ones[h * 32:h * 32 + 1, :],
                                         rd[h * 32:h * 32 + 1, :],
                                         start=True, stop=True,
                                         tile_position=(h * 32, 0))
                        nc.vector.tensor_tensor(
                            concat[pb:pb + DK, ft, :], au[:, :], ps_b[:, :],
                            mybir.AluOpType.mult)
                    pending.append(norm_h)

            for ic in range(SC):
                concat = cp.tile([128, FT, 512], F32R, tag="concat")
                aus = []
                dn = smp.tile([128, 512], F32, tag="dn")
                nc.vector.memset(dn[:], 1.0)  # unused lanes stay finite
                for ft in range(FT):           # head pair, fully interleaved
                    ps_ap = atp.tile([128, 1024], F32, tag="at")
                    exs = [None] * ST
                    for jt in range(ST + 1):
                        if jt < ST:
                            ps_s = scp.tile([128, 1024], F32, tag="sc")
                            for hs in range(2):
                                pb = hs * DK
                                nc.tensor.matmul(
                                    ps_s[:, hs * 512:(hs + 1) * 512],
                                    KT[pb:pb + DK, ft,
                                       jt * 128:(jt + 1) * 128],
                                    QT[pb:pb + DK, ft,
                                       ic * 512:(ic + 1) * 512],
                                    start=True, stop=True,
                                )
                            ex = ep.tile([128, 1024], F32R, tag="exp")
                            nc.scalar.activation(
                                ex[:], ps_s[:],
                                mybir.ActivationFunctionType.Exp,
                                bias=bias_t[:], scale=1.0 / np.sqrt(DK))
                            exs[jt] = ex
                        # PV runs one step behind so a stalled PV never
                        # head-of-line-blocks the next score pair in the
                        # in-order PE queue.
                        if jt > 0:
                            pj = jt - 1
                            for hs in range(2):
                                nc.tensor.matmul(
                                    ps_ap[0:DK + 1, hs * 512:(hs + 1) * 512],
                                    VAUG[pj][:, ft * 2 + hs, :],
                                    exs[pj][:, hs * 512:(hs + 1) * 512],
                                    start=(pj == 0), stop=(pj == ST - 1),
                                )
                        if pending and jt % 2 == 1:
                            pending.pop(0)()
                    # free the attention psum quickly: copy attn rows and the
                    # denominator rows (to 32-aligned partitions of dn).
                    for hs in range(2):
                        au = aup.tile([DK, 512], F32, tag="au")
                        nc.vector.tensor_copy(
                            au[:], ps_ap[0:DK, hs * 512:(hs + 1) * 512])
                        aus.append(au)
                        dpb = (ft * 2 + hs) * 32
                        nc.vector.tensor_copy(
                            dn[dpb:dpb + 1, :],
                            ps_ap[DK:DK + 1, hs * 512:(hs + 1) * 512])

                normalize(ic, concat, aus, dn)
                phase_c(ic, concat)

            for emit in pending:
                emit()

    nc.compile()
    return nc


_PROGRAM = None


def _get_program() -> bass.Bass:
    global _PROGRAM
    if _PROGRAM is None:
        _PROGRAM = _build_program()
    return _PROGRAM


def _prepare_in_maps(x, Wq, Wk, Wv, Wo):
    x = np.asarray(x, dtype=np.float32)
    Wq = np.asarray(Wq, dtype=np.float32)
    Wk = np.asarray(Wk, dtype=np.float32)
    Wv = np.asarray(Wv, dtype=np.float32)
    Wo = np.asarray(Wo, dtype=np.float32)
    in_maps = []
    for c in range(NCORES):
        b, hg = c // HGROUPS, c % HGROUPS
        rows = slice(hg * FH, (hg + 1) * FH)
        in_maps.append({
            "xt": _round_f32r(np.ascontiguousarray(x[b].T)),
            "wqt": _round_f32r(np.ascontiguousarray(Wq[rows, :].T)),
            "wkt": _round_f32r(np.ascontiguousarray(Wk[rows, :].T)),
            "wvt": _round_f32r(np.ascontiguousarray(Wv[rows, :].T)),
            "wot": _round_f32r(np.ascontiguousarray(Wo[:, rows].T)),
            "ones": np.ones((128, DK), np.float32),
        })
    return in_maps


def run(inputs: dict, **spmd_kwargs):
    """Run on all 8 cores; returns (full output, BassKernelResults)."""
    nc = _get_program()
    in_maps = _prepare_in_maps(**inputs)
    res = run_bass_kernel_spmd(nc, in_maps, core_ids=list(range(NCORES)),
                               **spmd_kwargs)
    partials = [r["y"] for r in res.results]
    out = np.empty((B, S, E), dtype=np.float32)
    for b in range(B):
        acc = partials[b * HGROUPS].astype(np.float32, copy=True)
        for hg in range(1, HGROUPS):
            acc += partials[b * HGROUPS + hg]
        out[b] = acc
    return out, res


def kernel(**inputs) -> np.ndarray:
    out, _ = run(inputs)
    return out


# revision 23
# speedup vs baseline: 1.0924x; 1.0924x over previous
"""Multi-head attention (B=2, S=2048, E=1024, H=16) on 8 Trainium2 NeuronCores.

Sharding: data-parallel over the 2 batches x tensor-parallel over 4 head-groups
(4 heads each).  Core c handles batch c//4, heads [4*(c%4), 4*(c%4)+4).
Each core computes its heads' Q/K/V projections, softmax(QK^T/8)V, and the
partial output projection against the matching Wo column slice; the host sums
the 4 partials per batch (the head-parallel all-reduce) and stacks batches.

Device-side layout notes:
 - Host pre-transposes x and the weight slices so every matmul operand already
   has its contraction dim on SBUF partitions (no on-device transposes).
 - Scores are produced transposed, sT[j, i] = k_j . q_i, so softmax(j) is a
   partition-dim reduction folded into the P@V matmul via a ones column on V
   (out row 64 = sum_j exp(sT[j, i])), and the attention output lands directly
   in the [head_dim, seq] layout the output projection needs as lhsT.
 - exp runs on the scalar engine straight out of PSUM with the 1/sqrt(dk)
   scale and a constant -4 bias folded in (softmax is shift-invariant).
 - Matmuls use float32r (full PE rate at free-dim >= 256, ~19-bit mantissa).
"""

import numpy as np
import ml_dtypes

import concourse.bass as bass
from concourse import bacc
import concourse.mybir as mybir
import concourse.tile as tile
from concourse.bass_utils import run_bass_kernel_spmd

B, S, E, H = 2, 2048, 1024, 16
DK = 64
NCORES = 8
HGROUPS = 4            # head-parallel groups per batch
HLOC = H // HGROUPS    # heads per core = 4
FH = HLOC * DK         # local feature cols = 256

F32 = mybir.dt.float32
F32R = mybir.dt.float32r
EXP_BIAS = -4.0        # constant shift inside exp; cancels in softmax


def _round_f32r(a: np.ndarray) -> np.ndarray:
    """Round fp32 to the bf16-pair (hi+lo) values the PE's fp32r mode uses."""
    hi = a.astype(ml_dtypes.bfloat16).astype(np.float32)
    lo = (a - hi).astype(ml_dtypes.bfloat16).astype(np.float32)
    return hi + lo


def _build_program() -> bass.Bass:
    nc = bacc.Bacc("TRN2", target_bir_lowering=False, debug=False,
                   enable_asserts=False)

    xt_d = nc.dram_tensor("xt", [E, S], F32R, kind="ExternalInput").ap()
    wqt_d = nc.dram_tensor("wqt", [E, FH], F32R, kind="ExternalInput").ap()
    wkt_d = nc.dram_tensor("wkt", [E, FH], F32R, kind="ExternalInput").ap()
    wvt_d = nc.dram_tensor("wvt", [E, FH], F32R, kind="ExternalInput").ap()
    wot_d = nc.dram_tensor("wot", [FH, E], F32R, kind="ExternalInput").ap()
    ones_d = nc.dram_tensor("ones", [128, DK], F32R, kind="ExternalInput").ap()
    y_d = nc.dram_tensor("y", [S, E], F32, kind="ExternalOutput").ap()

    EC = E // 128        # 8 contraction chunks for the projections
    ST = S // 128        # 16 seq tiles of 128 (the j tiles)
    SC = S // 512        # 4 seq chunks of 512 (the i chunks)
    FT = FH // 128       # 2 feature tiles (head pairs)

    with tile.TileContext(nc) as tc:
        with (
            tc.tile_pool(name="constp", bufs=1) as constp,
            tc.tile_pool(name="xtp", bufs=1) as xtp,
            tc.tile_pool(name="wp", bufs=1) as wp,
            tc.tile_pool(name="qkp", bufs=1) as qkp,
            tc.tile_pool(name="vp", bufs=ST) as vp,
            tc.tile_pool(name="cp", bufs=3) as cp,
            tc.tile_pool(name="ep", bufs=5) as ep,
            tc.tile_pool(name="aup", bufs=6) as aup,
            tc.tile_pool(name="smp", bufs=2) as smp,
            tc.tile_pool(name="op", bufs=2) as op,
            tc.tile_pool(name="mmp", bufs=2, space="PSUM") as mmp,
            tc.tile_pool(name="scp", bufs=2, space="PSUM") as scp,
            tc.tile_pool(name="atp", bufs=1, space="PSUM") as atp,
        ):
            ones = constp.tile([128, DK], F32R, tag="ones")
            nc.sync.dma_start(ones[:], ones_d)
            bias_t = constp.tile([128, 1], F32, tag="bias")
            nc.vector.memset(bias_t[:], EXP_BIAS)
            onescol = constp.tile([128, HLOC], F32, tag="onescol")
            nc.vector.memset(onescol[:], 1.0)

            # ---- input DMAs ----
            XT = xtp.tile([128, EC, S], F32R, tag="xt")
            xt_r = xt_d.rearrange("(c p) s -> p c s", p=128)
            for ec in range(EC):
                nc.sync.dma_start(XT[:, ec, :], xt_r[:, ec, :])

            WQ = wp.tile([128, EC, FH], F32R, tag="wq")
            WK = wp.tile([128, EC, FH], F32R, tag="wk")
            WV = wp.tile([128, EC, FH], F32R, tag="wv")
            WO = wp.tile([128, FT, E], F32R, tag="wo")
            nc.sync.dma_start(WK[:], wkt_d.rearrange("(c p) f -> p c f", p=128))
            nc.sync.dma_start(WQ[:], wqt_d.rearrange("(c p) f -> p c f", p=128))
            nc.sync.dma_start(WV[:], wvt_d.rearrange("(c p) f -> p c f", p=128))
            nc.sync.dma_start(WO[:], wot_d.rearrange("(c p) e -> p c e", p=128))

            # ---- PE warmup during the input-DMA window ----
            # The HAM clock gate starts at half clock and needs ~3.4us of
            # sustained PE activity; burn cheap bf16 matmuls on junk data
            # while the 13MB of inputs stream in so the projections run warm.
            warm = constp.tile([128, 512], mybir.dt.bfloat16, tag="warm")
            nc.vector.memset(warm[:], 1.0)
            ps_w = mmp.tile([128, 512], F32, tag="mm", name="warmps")
            for _ in range(130):
                nc.tensor.matmul(ps_w[:, :], warm[:, 0:128], warm[:, :],
                                 start=True, stop=True)

            # ---- projections ----
            # qT/kT: [f, s] layout.  out[f_tile, s_chunk] = sum_ec WqT^T @ xT
            # Order: k/q for head-pair 0, then v, then k/q for pair 1, so the
            # attention loop can start as soon as pair 0 and v are ready.
            QT = qkp.tile([128, FT, S], F32R, tag="qt")
            KT = qkp.tile([128, FT, S], F32R, tag="kt")

            def qk_proj(dst, w, ft):
                for sc in range(SC):
                    ps = mmp.tile([128, 512], F32, tag="mm", name="qkps")
                    for ec in range(EC):
                        nc.tensor.matmul(
                            ps[:, :],
                            w[:, ec, ft * 128:(ft + 1) * 128],
                            XT[:, ec, sc * 512:(sc + 1) * 512],
                            start=(ec == 0), stop=(ec == EC - 1),
                        )
                    nc.vector.tensor_copy(
                        dst[:, ft, sc * 512:(sc + 1) * 512], ps[:, :])

            # v: natural [s, f] layout, plus a fused ones column per head:
            # VAUG[jt] is [128, HLOC, DK+1] with [:, h, DK] == 1.
            VAUG = []

            def v_proj():
                for jt in range(ST):
                    va = vp.tile([128, HLOC, DK + 1], F32R, tag="vaug")
                    nc.vector.tensor_copy(va[:, :, DK:DK + 1],
                                          onescol[:, :, None])
                    ps = mmp.tile([128, 512], F32, tag="mm", name="vps")
                    for ec in range(EC):
                        nc.tensor.matmul(
                            ps[:, 0:FH],
                            XT[:, ec, jt * 128:(jt + 1) * 128],
                            WV[:, ec, :],
                            start=(ec == 0), stop=(ec == EC - 1),
                        )
                    nc.vector.tensor_copy(
                        va[:, :, 0:DK],
                        ps[:, 0:FH].rearrange("p (h d) -> p h d", d=DK))
                    VAUG.append(va)

            qk_proj(KT, WK, 0)
            qk_proj(QT, WQ, 0)
            v_proj()

            # ---- attention + output projection, per 512-wide i chunk ----
            # Heads are processed in pairs (partition bases 0/64) so the K=64
            # score matmuls run concurrently in distinct PE row groups.  The
            # attention PSUM is copied to SBUF right after the PV chain so the
            # slow single-partition reciprocal never holds a PSUM bank (PE
            # gaps at head boundaries re-throttle the HAM clock gate).
            # Output projection for chunk ic is emitted interleaved into the
            # NEXT chunk's jt loops so the PE never bursts at boundaries and
            # the scalar engine keeps receiving fresh score tiles.
            pending = []
            for _dst, _w, _ft in ((KT, WK, 1), (QT, WQ, 1)):
                for _sc in range(SC):
                    def qk_item(dst=_dst, w=_w, ft=_ft, sc=_sc):
                        ps = mmp.tile([128, 512], F32, tag="mm", name="qkps")
                        for ec in range(EC):
                            nc.tensor.matmul(
                                ps[:, :],
                                w[:, ec, ft * 128:(ft + 1) * 128],
                                XT[:, ec, sc * 512:(sc + 1) * 512],
                                start=(ec == 0), stop=(ec == EC - 1),
                            )
                        nc.vector.tensor_copy(
                            dst[:, ft, sc * 512:(sc + 1) * 512], ps[:, :])
                    pending.append(qk_item)

            def phase_c(ic, concat):
                for stl in range(4):
                    st = ic * 4 + stl
                    for oc in range(2):
                        def emit(st=st, oc=oc, stl=stl, concat=concat):
                            ps_o = mmp.tile([128, 512], F32, tag="mm",
                                            name="ops")
                            for fc in range(FT):
                                nc.tensor.matmul(
                                    ps_o[:, :],
                                    concat[:, fc, stl * 128:(stl + 1) * 128],
                                    WO[:, fc, oc * 512:(oc + 1) * 512],
                                    start=(fc == 0), stop=(fc == FT - 1),
                                )
                            ot = op.tile([128, 512], F32, tag="out")
                            nc.vector.tensor_copy(ot[:], ps_o[:, :])
                            nc.sync.dma_start(
                                y_d[st * 128:(st + 1) * 128,
                                    oc * 512:(oc + 1) * 512],
                                ot[:])
                        pending.append(emit)

            def normalize(ic, concat, aus, dn):
                # batched reciprocal for all 4 heads of chunk ic, then four
                # broadcast-matmul + multiply pairs writing concat.
                rd = smp.tile([128, 512], F32R, tag="rd")

                def recip(rd=rd, dn=dn):
                    with nc.allow_low_precision(
                            reason="f32r softmax denominators"):
                        nc.vector.reciprocal(rd[:], dn[:])
                pending.append(recip)

                for h in range(HLOC):
                    def norm_h(h=h, rd=rd, concat=concat, au=aus[h]):
                        ft, hs = h // 2, h % 2
                        pb = hs * DK
                        ps_b = mmp.tile([DK, 512], F32, tag="mm", name="bc")
                        nc.tensor.matmul(ps_b[:, :],
                                         ones[h * 32:h * 32 + 1, :],
                                         rd[h * 32:h * 32 + 1, :],
                                         start=True, stop=True,
                                         tile_position=(h * 32, 0))
                        nc.vector.tensor_tensor(
                            concat[pb:pb + DK, ft, :], au[:, :], ps_b[:, :],
                            mybir.AluOpType.mult)
                    pending.append(norm_h)

            for ic in range(SC):
                concat = cp.tile([128, FT, 512], F32R, tag="concat")
                aus = []
                dn = smp.tile([128, 512], F32, tag="dn")
                nc.vector.memset(dn[:], 1.0)  # unused lanes stay finite
                for ft in range(FT):           # head pair, fully interleaved
                    ps_ap = atp.tile([128, 1024], F32, tag="at")
                    for jt in range(ST):
                        ps_s = scp.tile([128, 1024], F32, tag="sc")
                        for hs in range(2):
                            pb = hs * DK
                            nc.tensor.matmul(
                                ps_s[:, hs * 512:(hs + 1) * 512],
                                KT[pb:pb + DK, ft, jt * 128:(jt + 1) * 128],
                                QT[pb:pb + DK, ft, ic * 512:(ic + 1) * 512],
                                start=True, stop=True,
                            )
                        ex = ep.tile([128, 1024], F32R, tag="exp")
                        nc.scalar.activation(
                            ex[:], ps_s[:], mybir.ActivationFunctionType.Exp,
                            bias=bias_t[:], scale=1.0 / np.sqrt(DK))
                        for hs in range(2):
                            nc.tensor.matmul(
                                ps_ap[0:DK + 1, hs * 512:(hs + 1) * 512],
                                VAUG[jt][:, ft * 2 + hs, :],
                                ex[:, hs * 512:(hs + 1) * 512],
                                start=(jt == 0), stop=(jt == ST - 1),
                            )
                        if pending and jt % 2 == 1:
                            pending.pop(0)()
                    # free the attention psum quickly: copy attn rows and the
                    # denominator rows (to 32-aligned partitions of dn).
                    for hs in range(2):
                        au = aup.tile([DK, 512], F32, tag="au")
                        nc.vector.tensor_copy(
                            au[:], ps_ap[0:DK, hs * 512:(hs + 1) * 512])
                        aus.append(au)
                        dpb = (ft * 2 + hs) * 32
                        nc.vector.tensor_copy(
                            dn[dpb:dpb + 1, :],
                            ps_ap[DK:DK + 1, hs * 512:(hs + 1) * 512])

                normalize(ic, concat, aus, dn)
                phase_c(ic, concat)

            for emit in pending:
                emit()

    nc.compile()
    return nc


_PROGRAM = None


def _get_program() -> bass.Bass:
    global _PROGRAM
    if _PROGRAM is None:
        _PROGRAM = _build_program()
    return _PROGRAM


def _prepare_in_maps(x, Wq, Wk, Wv, Wo):
    x = np.asarray(x, dtype=np.float32)
    Wq = np.asarray(Wq, dtype=np.float32)
    Wk = np.asarray(Wk, dtype=np.float32)
    Wv = np.asarray(Wv, dtype=np.float32)
    Wo = np.asarray(Wo, dtype=np.float32)
    in_maps = []
    for c in range(NCORES):
        b, hg = c // HGROUPS, c % HGROUPS
        rows = slice(hg * FH, (hg + 1) * FH)
        in_maps.append({
            "xt": _round_f32r(np.ascontiguousarray(x[b].T)),
            "wqt": _round_f32r(np.ascontiguousarray(Wq[rows, :].T)),
            "wkt": _round_f32r(np.ascontiguousarray(Wk[rows, :].T)),
            "wvt": _round_f32r(np.ascontiguousarray(Wv[rows, :].T)),
            "wot": _round_f32r(np.ascontiguousarray(Wo[:, rows].T)),
            "ones": np.ones((128, DK), np.float32),
        })
    return in_maps


def run(inputs: dict, **spmd_kwargs):
    """Run on all 8 cores; returns (full output, BassKernelResults)."""
    nc = _get_program()
    in_maps = _prepare_in_maps(**inputs)
    res = run_bass_kernel_spmd(nc, in_maps, core_ids=list(range(NCORES)),
                               **spmd_kwargs)
    partials = [r["y"] for r in res.results]
    out = np.empty((B, S, E), dtype=np.float32)
    for b in range(B):
        acc = partials[b * HGROUPS].astype(np.float32, copy=True)
        for hg in range(1, HGROUPS):
            acc += partials[b * HGROUPS + hg]
        out[b] = acc
    return out, res


def kernel(**inputs) -> np.ndarray:
    out, _ = run(inputs)
    return out
